# revision 1
# baseline (speedup 1.0000x reference)
"""SE(3) attention block (GNN message passing) on 8 Trainium2 NeuronCores.

Strategy
--------
Edges are sorted by destination node on the host. Nodes are cut into tiles of
(<=128 nodes, <=2048 edges); every tile's edges are padded to exactly 2048
slots (16 blocks of 128 edges). Tiles are distributed contiguously across the
8 cores, so every (node, head) softmax group lives entirely on one core and
inside one tile -> no cross-device collectives at all. The destination-node
query vector for each edge slot is pre-gathered on the host (sorted edges ->
a pure layout transform) and shipped transposed next to k^T.

Per node tile the device kernel:
  1. builds the one-hot edge->local-node matrix [e, n] with is_equal
     compares on DVE (one half via an ACT-widened dense dst map in 2x
     mode, one half straight from broadcast per-block dst scalars),
  2. prodT = kT * qgT elementwise (one whole-tile DVE op, bf16 2x),
  3. per-head scores via 16 head-mask matmuls (N=8) into one PSUM bank,
  4. one exp over the tile's [128, 128] scores (ACT, bf16 out),
  5. widens ex to the interleaved 17-stride (ACT) and forms the
     [ex | ex*v] scatter rhs with one whole-tile DVE 2x multiply
     (v is sent from the host with a 1.0 column per head: 17 cols/head),
  6. 16 back-to-back scatter-add matmuls (bf16, N=136) accumulate into a
     [128, 136] PSUM tile.
The tile is then normalized by 1/sum(exp) (strided APs pull ssum/agg out
of the interleaved accumulator) and written out. The host scatters
per-tile rows back into the full [N, 32, 4] output. GPSIMD is left idle
on purpose: concurrent GPSIMD tensor ops contend with DVE for SBUF ports
and inflate both by ~50%.
"""

import math
import numpy as np

# ---------------------------------------------------------------- constants
N_CORES = 8
P = 128                 # partitions / nodes per tile / edges per block
F_BLOCKS = 16           # edge blocks per node tile
EPT = F_BLOCKS * P      # edge slots per tile (2048)
T_PC = 50               # node tiles per core (max 400 total; ~395 needed)
H = 8                   # heads
NF = 128                # features per edge (32*4)
HS = NF // H            # head size (16)
HS1 = HS + 1            # interleaved head stride (ex + 16 features)
N_NODES = 50000
E_EDGES = 800000
PAD_DST = 300.0         # local-dst sentinel for padding edge slots
INV_SQRT_NF = 1.0 / math.sqrt(NF)

_CACHE = {}
LAST_RESULTS = None     # BassKernelResults of the most recent run (for test.py)


# ---------------------------------------------------------------- device IR
def build_nc(tpc=T_PC, f_blocks=F_BLOCKS, v_bf16=True):
    """Build the per-core Bass/Tile program (identical on all 8 cores)."""
    from contextlib import ExitStack

    import concourse.bacc as bacc
    import concourse.mybir as mybir
    from concourse.tile import TileContext

    f32 = mybir.dt.float32
    bf16 = mybir.dt.bfloat16
    vdt = bf16 if v_bf16 else f32
    ept = f_blocks * P

    nc = bacc.Bacc("TRN2", target_bir_lowering=False, debug=False)
    kq_d = nc.dram_tensor("kq", [tpc, P, 2 * ept], bf16, kind="ExternalInput")
    vd_d = nc.dram_tensor("vd", [tpc, P, f_blocks * (H * HS1 + 1)], vdt,
                          kind="ExternalInput")
    io_d = nc.dram_tensor("iota", [P, ept], bf16, kind="ExternalInput")
    hm_d = nc.dram_tensor("hm", [P, H], bf16, kind="ExternalInput")
    out_d = nc.dram_tensor("out", [tpc, P, P], f32, kind="ExternalOutput")

    with TileContext(nc) as tc, ExitStack() as ctx:
        singles = ctx.enter_context(tc.tile_pool(name="singles", bufs=1))
        big = ctx.enter_context(tc.tile_pool(name="big", bufs=5))
        med = ctx.enter_context(tc.tile_pool(name="med", bufs=3))
        sml = ctx.enter_context(tc.tile_pool(name="sml", bufs=4))
        ps_sc = ctx.enter_context(tc.tile_pool(name="ps_sc", bufs=5, space="PSUM"))
        ps_ag = ctx.enter_context(tc.tile_pool(name="ps_ag", bufs=3, space="PSUM"))

        iota_wide_sb = singles.tile([P, ept], bf16)
        nc.sync.dma_start(out=iota_wide_sb[:], in_=io_d[:, :])
        hm_sb = singles.tile([P, H], bf16)
        nc.sync.dma_start(out=hm_sb[:], in_=hm_d[:, :])

        for t in range(tpc):
            kq_sb = big.tile([P, 2 * ept], bf16, tag="kq")
            nc.sync.dma_start(out=kq_sb[:], in_=kq_d[t])
            kt_sb = kq_sb[:, 0:ept]
            qg_sb = kq_sb[:, ept:2 * ept]
            vd_sb = big.tile([P, f_blocks * (H * HS1 + 1)], vdt, tag="vd")
            nc.sync.dma_start(out=vd_sb[:], in_=vd_d[t])
            v_sb = vd_sb[:, 0:f_blocks * H * HS1]
            dl_sb = vd_sb[:, f_blocks * H * HS1:]

            agg_ps = ps_ag.tile([P, H * HS1], f32, tag="agg")

            hb = f_blocks // 2
            oh_en = med.tile([P, ept], bf16, tag="oh_en", bufs=4)
            prodT = med.tile([P, ept], bf16, tag="prodT", bufs=3)
            c0 = slice(0, hb * P)
            c1 = slice(hb * P, ept)
            # half 0: ACT-widened dst map + 2x DVE compare
            dlw = med.tile([P, hb * P], bf16, tag="dlw", bufs=2)
            nc.scalar.copy(
                out=dlw[:].rearrange("p (b n) -> p b n", b=hb),
                in_=dl_sb[:, 0:hb].to_broadcast([P, hb, P]),
            )
            nc.vector.tensor_tensor(
                out=oh_en[:, c0], in0=iota_wide_sb[:, c0], in1=dlw[:],
                op=mybir.AluOpType.is_equal,
            )
            # half 1: direct broadcast compare on DVE (1x)
            nc.vector.tensor_tensor(
                out=oh_en[:, c1].rearrange("p (b n) -> p b n", b=hb),
                in0=iota_wide_sb[:, c1].rearrange("p (b n) -> p b n", b=hb),
                in1=dl_sb[:, hb:f_blocks].to_broadcast([P, hb, P]),
                op=mybir.AluOpType.is_equal,
            )
            # prodT[f, e] = kT * qgT (DVE, bf16 2x)
            nc.vector.tensor_tensor(
                out=prodT[:], in0=kt_sb[:], in1=qg_sb[:],
                op=mybir.AluOpType.mult,
            )
            # per-head scores for all 16 blocks into one PSUM bank
            sc_ps = ps_sc.tile([P, f_blocks * H], f32, tag="sc")
            for b in range(f_blocks):
                nc.tensor.matmul(
                    out=sc_ps[:, b * H:(b + 1) * H],
                    lhsT=prodT[:, b * P:(b + 1) * P], rhs=hm_sb[:],
                    start=True, stop=True,
                )
            # ex = exp(score / sqrt(nf)) for the whole tile (one ACT op)
            ex_t = sml.tile([P, f_blocks * H], bf16, tag="ex")
            nc.scalar.activation(
                out=ex_t[:], in_=sc_ps[:],
                func=mybir.ActivationFunctionType.Exp,
                scale=INV_SQRT_NF,
            )
            # widen ex to the interleaved stride (one ACT op), then one
            # whole-tile DVE 2x multiply for [ex | ex*v]
            ex_w = med.tile([P, f_blocks * H * HS1], bf16, tag="ex_w", bufs=3)
            evex = med.tile([P, f_blocks * H * HS1], bf16, tag="evex", bufs=3)
            hw = f_blocks * H * HS1 // 2
            for h2 in range(2):
                wcols = slice(h2 * hw, (h2 + 1) * hw)
                nc.scalar.copy(
                    out=ex_w[:, wcols].rearrange("p (x s) -> p x s", s=HS1),
                    in_=ex_t[:, h2 * f_blocks * H // 2:
                             (h2 + 1) * f_blocks * H // 2].to_broadcast(
                        [P, f_blocks * H // 2, HS1]),
                )
            nc.vector.tensor_tensor(
                out=evex[:], in0=v_sb[:], in1=ex_w[:],
                op=mybir.AluOpType.mult,
            )
            # scatter-add all 16 blocks back-to-back (dense PE burst)
            for b in range(f_blocks):
                nc.tensor.matmul(
                    out=agg_ps[:],
                    lhsT=oh_en[:, b * P:(b + 1) * P],
                    rhs=evex[:, b * H * HS1:(b + 1) * H * HS1],
                    start=(b == 0), stop=(b == f_blocks - 1),
                )
            # normalize: out[n, f] = agg[n, f] / ssum[n, h(f)]
            agg_v = agg_ps[:].rearrange("p (h s) -> p h s", h=H)
            inv = sml.tile([P, H], f32, tag="inv")
            nc.vector.tensor_scalar(
                out=inv[:], in0=agg_v[:, :, 0],
                scalar1=1e-30, scalar2=None, op0=mybir.AluOpType.add,
            )
            nc.vector.reciprocal(out=inv[:], in_=inv[:])
            out_sb = med.tile([P, P], f32, tag="out")
            nc.vector.tensor_tensor(
                out=out_sb[:].rearrange("p (h j) -> p h j", h=H),
                in0=agg_v[:, :, 1:],
                in1=inv[:].to_broadcast([P, H, HS]),
                op=mybir.AluOpType.mult,
            )
            nc.sync.dma_start(out=out_d[t], in_=out_sb[:])
    nc.compile()
    return nc


# ------------------------------------------------------------ host plumbing
def _build_tiles(cum, n_nodes, ept):
    """Greedy cut of nodes into (<=128 nodes, <=ept edges) tiles."""
    tiles = []
    n0 = 0
    while n0 < n_nodes:
        n1 = int(np.searchsorted(cum, cum[n0] + ept, side="right")) - 1
        n1 = min(n1, n0 + P, n_nodes)
        if n1 <= n0:
            raise ValueError(f"node {n0} has degree > {ept}; unsupported")
        tiles.append((n0, n1))
        n0 = n1
    return tiles


def _prep_inputs(value, key, query_0, query_1, edge_index,
                 tpc=T_PC, f_blocks=F_BLOCKS, n_cores=N_CORES, v_bf16=True):
    """Sort/tile/pad on the host; returns per-core input maps + assembly info."""
    import ml_dtypes
    bf16 = ml_dtypes.bfloat16

    ept = f_blocks * P
    value = np.ascontiguousarray(np.asarray(value, dtype=np.float32))
    key = np.ascontiguousarray(np.asarray(key, dtype=np.float32))
    q0 = np.asarray(query_0, dtype=np.float32)
    q1 = np.asarray(query_1, dtype=np.float32)
    ei = np.asarray(edge_index)
    n_nodes = q0.shape[0]
    n_edges = key.shape[0]

    dst = ei[1].astype(np.int64).ravel()
    order = np.argsort(dst, kind="stable")
    dsts = dst[order]
    counts = np.bincount(dsts, minlength=n_nodes)
    cum = np.zeros(n_nodes + 1, np.int64)
    cum[1:] = np.cumsum(counts)

    tiles = _build_tiles(cum, n_nodes, ept)
    t_total = len(tiles)
    if t_total > n_cores * tpc:
        raise ValueError(f"{t_total} tiles > capacity {n_cores * tpc}")
    q_per_core = (t_total + n_cores - 1) // n_cores  # real tiles per core
    t8 = n_cores * tpc

    # slot -> original edge id (or padding), per global tile slot
    slot_edge = np.full((t8, ept), 0, np.int64)
    slot_valid = np.zeros((t8, ept), bool)
    slot_dst = np.full((t8, ept), 0, np.int64)   # global dst per slot
    dl = np.full((t8, ept), PAD_DST, np.float32)
    tile_info = []  # (global_tile_idx, n0, n_cnt)
    for i, (n0, n1) in enumerate(tiles):
        c, j = divmod(i, q_per_core)
        idx = c * tpc + j
        e0, e1 = int(cum[n0]), int(cum[n1])
        cnt = e1 - e0
        slot_edge[idx, :cnt] = order[e0:e1]
        slot_valid[idx, :cnt] = True
        slot_dst[idx, :cnt] = dsts[e0:e1]
        dl[idx, :cnt] = (dsts[e0:e1] - n0).astype(np.float32)
        tile_info.append((idx, n0, n1 - n0))

    flat_edge = slot_edge.reshape(-1)
    flat_valid = slot_valid.reshape(-1)

    kf = key.reshape(n_edges, NF)
    k_slots = kf[flat_edge]
    k_slots[~flat_valid] = 0.0
    q_cat = np.concatenate([q0, q1], axis=-1).reshape(
        n_nodes, NF).astype(np.float32)
    qg_slots = q_cat[slot_dst.reshape(-1)]
    qg_slots[~flat_valid] = 0.0
    # merged [kT | qgT]: [t, f, b*128+e] twice, bf16
    kq = np.empty((t8, NF, 2 * ept), bf16)
    kq[:, :, :ept] = k_slots.reshape(
        t8, f_blocks, P, NF).transpose(0, 3, 1, 2).reshape(t8, NF, ept)
    kq[:, :, ept:] = qg_slots.reshape(
        t8, f_blocks, P, NF).transpose(0, 3, 1, 2).reshape(t8, NF, ept)
    del k_slots, qg_slots

    vf = value.reshape(n_edges, NF)
    v_slots = vf[flat_edge]
    v_slots[~flat_valid] = 0.0
    # interleaved v17: [t, e, b, h, 1+16] with a leading 1.0 per head,
    # then the per-block local-dst columns appended: [t, e, b]
    v17 = np.empty((t8, f_blocks, P, H, HS1), np.float32)
    v17[..., 0] = 1.0
    v17[..., 1:] = v_slots.reshape(t8, f_blocks, P, H, HS)
    del v_slots
    vd = np.empty((t8, P, f_blocks * (H * HS1 + 1)), np.float32)
    vd[:, :, :f_blocks * H * HS1] = v17.transpose(0, 2, 1, 3, 4).reshape(
        t8, P, f_blocks * H * HS1)
    del v17
    vd[:, :, f_blocks * H * HS1:] = dl.reshape(
        t8, f_blocks, P).transpose(0, 2, 1)
    vd = vd.astype(bf16) if v_bf16 else vd

    iota = np.broadcast_to(np.arange(P, dtype=np.float32)[None, None, :],
                           (P, F_BLOCKS, P)).reshape(P, F_BLOCKS * P).astype(bf16)
    hm = np.zeros((NF, H), np.float32)
    for h in range(H):
        hm[h * HS:(h + 1) * HS, h] = 1.0
    hm = hm.astype(bf16)

    in_maps = []
    for c in range(n_cores):
        s = slice(c * tpc, (c + 1) * tpc)
        in_maps.append({
            "kq": kq[s], "vd": vd[s], "iota": iota, "hm": hm,
        })
    return in_maps, tile_info, n_nodes


def _assemble(results, tile_info, n_nodes, tpc=T_PC):
    out = np.zeros((n_nodes, NF), np.float32)
    for idx, n0, cnt in tile_info:
        c, j = divmod(idx, tpc)
        out[n0:n0 + cnt] = results[c]["out"][j, :cnt]
    return out.reshape(n_nodes, NF // 4, 4)


def _get_nc(tpc=T_PC, f_blocks=F_BLOCKS, v_bf16=True):
    key = (tpc, f_blocks, v_bf16)
    if key not in _CACHE:
        _CACHE[key] = build_nc(tpc, f_blocks, v_bf16)
    return _CACHE[key]


def _needed_tpc(edge_index, n_nodes, ept, n_cores=N_CORES):
    dst = np.asarray(edge_index)[1].astype(np.int64).ravel()
    counts = np.bincount(dst, minlength=n_nodes)
    cum = np.zeros(n_nodes + 1, np.int64)
    cum[1:] = np.cumsum(counts)
    t_total = len(_build_tiles(cum, n_nodes, ept))
    return (t_total + n_cores - 1) // n_cores


def _run(inputs, trace=False, tpc=T_PC, f_blocks=F_BLOCKS, v_bf16=True,
         **spmd_kwargs):
    global LAST_RESULTS
    from concourse.bass_utils import run_bass_kernel_spmd

    tpc = max(tpc, _needed_tpc(inputs["edge_index"],
                               np.asarray(inputs["query_0"]).shape[0],
                               f_blocks * P))
    nc = _get_nc(tpc, f_blocks, v_bf16)
    in_maps, tile_info, n_nodes = _prep_inputs(
        inputs["value"], inputs["key"], inputs["query_0"], inputs["query_1"],
        inputs["edge_index"], tpc=tpc, f_blocks=f_blocks, v_bf16=v_bf16)
    res = run_bass_kernel_spmd(
        nc, in_maps, list(range(N_CORES)), trace=trace, **spmd_kwargs)
    LAST_RESULTS = res
    return _assemble(res.results, tile_info, n_nodes, tpc=tpc)


def kernel(value, key, query_0, query_1, edge_index):
    return _run({
        "value": value, "key": key, "query_0": query_0,
        "query_1": query_1, "edge_index": edge_index,
    })



# revision 3
# speedup vs baseline: 1.0576x; 1.0576x over previous
"""SE(3) attention block (GNN message passing) on 8 Trainium2 NeuronCores.

Strategy (slot format, v3)
--------------------------
Nodes are sorted by in-degree (host) and cut into tiles of 128 nodes.
Tiles are grouped 8 at a time (one tile per core, SPMD) and every tile in
a group is padded to the group's max degree S, so all 8 cores execute an
identical per-tile shape profile.  Because nodes in a tile have nearly
equal degree (sorted), slot padding is only ~2-3% of E.

Each node-row owns its incoming edges as "slots" 0..S-1, so the segment
softmax and the weighted aggregation become *free-axis* operations on the
node-partitioned tile -- no one-hot matrices, no per-edge gathered query,
no cross-device collectives:

  1. prodT[f, (s, n)] = kT * qT (DVE, bf16 2x; q broadcast over slots via
     a 0-stride middle AP dim -- queries ship once per node, not per edge)
  2. scores[n, (s, h)]: S head-mask matmuls (PE, otherwise idle)
  3. ex = exp(scores / sqrt(NF)) (ACT, straight from PSUM)
  4. ex_w: widen ex per head to 16 features (ACT copy, trailing bcast)
  5. evex[n, (s, f)] = v * ex_w (DVE, bf16 2x)
  6. agg[n, f]: pairwise-add slot tree (bf16 2x) + final strided
     reduce_sum in f32 (DVE)
  7. ssum[n, h] = reduce_sum over slots (f32), inv = 1/(ssum+eps)
  8. out = agg * inv (bf16 out)

Padding slots carry k_pad = -C * q_n / |q_n|^2 so their score is a large
negative constant (ex ~ 1e-13) and v_pad = 0 -- no masks needed.
"""

import math
import numpy as np

# ---------------------------------------------------------------- constants
N_CORES = 8
P = 128                 # partitions / nodes per tile
H = 8                   # heads
NF = 128                # features per edge (32*4)
HS = NF // H            # head size (16)
INV_SQRT_NF = 1.0 / math.sqrt(NF)
C_PAD = 345.0           # pad-slot score magnitude (scaled: ~-30.5)

_CACHE = {}
LAST_RESULTS = None     # BassKernelResults of the most recent run (for test.py)


# ---------------------------------------------------------------- device IR
def build_nc(s_prof):
    """Per-core Bass/Tile program; identical on all 8 cores (SPMD)."""
    from contextlib import ExitStack

    import concourse.bacc as bacc
    import concourse.mybir as mybir
    from concourse.tile import TileContext

    f32 = mybir.dt.float32
    bf16 = mybir.dt.bfloat16
    G = len(s_prof)
    W = int(sum(s_prof)) * P

    nc = bacc.Bacc("TRN2", target_bir_lowering=False, debug=False)
    kT_d = nc.dram_tensor("kT", [P, W], bf16, kind="ExternalInput")
    v_d = nc.dram_tensor("v", [P, W], bf16, kind="ExternalInput")
    qT_d = nc.dram_tensor("qT", [G, P, P], bf16, kind="ExternalInput")
    hm_d = nc.dram_tensor("hm", [P, H], bf16, kind="ExternalInput")
    out_d = nc.dram_tensor("out", [G, P, P], bf16, kind="ExternalOutput")

    with TileContext(nc) as tc, ExitStack() as ctx:
        singles = ctx.enter_context(tc.tile_pool(name="singles", bufs=1))
        inp = ctx.enter_context(tc.tile_pool(name="inp", bufs=4))
        mid = ctx.enter_context(tc.tile_pool(name="mid", bufs=3))
        sml = ctx.enter_context(tc.tile_pool(name="sml", bufs=4))
        ps = ctx.enter_context(tc.tile_pool(name="ps", bufs=6, space="PSUM"))

        hm = singles.tile([P, H], bf16)
        nc.sync.dma_start(out=hm[:], in_=hm_d[:, :])

        off = 0
        for g in range(G):
            S = int(s_prof[g])
            Wt = S * P
            kT = inp.tile([P, Wt], bf16, tag="kT")
            nc.sync.dma_start(out=kT[:], in_=kT_d[:, off:off + Wt])
            v = inp.tile([P, Wt], bf16, tag="v")
            nc.sync.dma_start(out=v[:], in_=v_d[:, off:off + Wt])
            qT = sml.tile([P, P], bf16, tag="qT")
            nc.sync.dma_start(out=qT[:], in_=qT_d[g])
            off += Wt

            # 1. prodT[f, (s, n)] = kT * qT (q broadcast over slots)
            prodT = mid.tile([P, Wt], bf16, tag="prodT")
            nc.vector.tensor_tensor(
                out=prodT[:].rearrange("p (s n) -> p s n", s=S),
                in0=kT[:].rearrange("p (s n) -> p s n", s=S),
                in1=qT[:, :].unsqueeze(1).broadcast_to([P, S, P]),
                op=mybir.AluOpType.mult)

            # 2. per-slot head-mask matmuls -> scores [n, (s, h)] in PSUM
            sc = ps.tile([P, S * H], f32, tag="sc")
            for s in range(S):
                nc.tensor.matmul(
                    out=sc[:, s * H:(s + 1) * H],
                    lhsT=prodT[:, s * P:(s + 1) * P], rhs=hm[:],
                    start=True, stop=True)

            # 3. ex = exp(scores / sqrt(NF))
            ex = sml.tile([P, S * H], bf16, tag="ex")
            nc.scalar.activation(
                out=ex[:], in_=sc[:],
                func=mybir.ActivationFunctionType.Exp, scale=INV_SQRT_NF)

            # 4. widen ex to per-feature (trailing broadcast on ACT)
            exw = mid.tile([P, Wt], bf16, tag="exw")
            nc.scalar.copy(
                out=exw[:].rearrange("p (s h j) -> p s h j", s=S, h=H),
                in_=ex[:].rearrange("p (s h) -> p s h", s=S)
                    .to_broadcast([P, S, H, HS]))

            # 5. evex = v * ex_w
            evex = mid.tile([P, Wt], bf16, tag="evex")
            nc.vector.tensor_tensor(
                out=evex[:], in0=v[:], in1=exw[:], op=mybir.AluOpType.mult)

            # 6. agg[n, f]: halve slots with a bf16 tree add, then strided
            # f32 reduce over the remaining slots
            red_src = evex
            Sr = S
            if S >= 8 and S % 2 == 0:
                half = mid.tile([P, (S // 2) * P], bf16, tag="half")
                nc.vector.tensor_tensor(
                    out=half[:], in0=evex[:, 0:(S // 2) * P],
                    in1=evex[:, (S // 2) * P:], op=mybir.AluOpType.add)
                red_src = half
                Sr = S // 2
            agg = sml.tile([P, P], f32, tag="agg")
            nc.vector.tensor_reduce(
                out=agg[:],
                in_=red_src[:, 0:Sr * P].rearrange("p (s f) -> p f s", s=Sr),
                axis=mybir.AxisListType.X, op=mybir.AluOpType.add)

            # 7. ssum / inv
            ssum = sml.tile([P, H], f32, tag="ssum")
            nc.vector.tensor_reduce(
                out=ssum[:], in_=ex[:].rearrange("p (s h) -> p h s", s=S),
                axis=mybir.AxisListType.X, op=mybir.AluOpType.add)
            inv = sml.tile([P, H], f32, tag="inv")
            nc.vector.tensor_scalar(
                out=inv[:], in0=ssum[:], scalar1=1e-30, scalar2=None,
                op0=mybir.AluOpType.add)
            nc.vector.reciprocal(out=inv[:], in_=inv[:])

            # 8. normalize -> bf16 out
            outb = sml.tile([P, P], bf16, tag="outb")
            nc.vector.tensor_tensor(
                out=outb[:].rearrange("p (h j) -> p h j", h=H),
                in0=agg[:].rearrange("p (h j) -> p h j", h=H),
                in1=inv[:].to_broadcast([P, H, HS]),
                op=mybir.AluOpType.mult)
            nc.sync.dma_start(out=out_d[g], in_=outb[:])
    nc.compile()
    return nc


# ------------------------------------------------------------ host plumbing
def _plan(edge_index, n_nodes):
    """Degree-sorted tile plan shared by all cores."""
    dst = np.asarray(edge_index)[1].astype(np.int64).ravel()
    n_edges = dst.shape[0]
    counts = np.bincount(dst, minlength=n_nodes)
    order_e = np.argsort(dst, kind="stable")
    cum = np.zeros(n_nodes + 1, np.int64)
    cum[1:] = np.cumsum(counts)
    nperm = np.argsort(-counts, kind="stable")

    n_tiles = -(-n_nodes // P)
    G = -(-n_tiles // N_CORES)
    rows_total = G * N_CORES * P
    rnode = np.full(rows_total, -1, np.int64)
    rnode[:n_nodes] = nperm

    deg_pad = np.zeros(rows_total, np.int64)
    deg_pad[:n_nodes] = counts[nperm]
    s_prof = deg_pad.reshape(G, N_CORES * P).max(axis=1)
    s_prof = np.maximum(s_prof, 1).astype(np.int64)
    return dict(counts=counts, order_e=order_e, cum=cum, rnode=rnode,
                s_prof=s_prof, G=G, n_edges=n_edges, n_nodes=n_nodes)


def _prep_inputs(value, key, query_0, query_1, plan):
    import ml_dtypes
    bf16 = ml_dtypes.bfloat16

    G = plan["G"]
    s_prof = plan["s_prof"]
    rnode = plan["rnode"]
    counts, order_e, cum = plan["counts"], plan["order_e"], plan["cum"]
    n_edges = plan["n_edges"]
    n_nodes = plan["n_nodes"]

    key_f = np.asarray(key, dtype=np.float32).reshape(n_edges, NF)
    val_f = np.asarray(value, dtype=np.float32).reshape(n_edges, NF)
    q_cat = np.concatenate(
        [np.asarray(query_0, np.float32), np.asarray(query_1, np.float32)],
        axis=-1).reshape(n_nodes, NF)
    # pad slots must score ~-C in EVERY head (scores are per-head dots over
    # 16 features), so normalize q per head-block
    qh = q_cat.reshape(n_nodes, H, HS)
    qh2 = np.einsum("nhj,nhj->nh", qh, qh)
    kpad = (-C_PAD * qh / np.maximum(qh2, 0.1)[:, :, None]).reshape(
        n_nodes, NF)

    W = int(s_prof.sum()) * P
    kT_all = np.empty((N_CORES, P, W), bf16)
    v_all = np.empty((N_CORES, P, W), bf16)
    qT_all = np.empty((N_CORES, G, P, P), bf16)

    off = 0
    RG = N_CORES * P       # rows per group
    for g in range(G):
        S = int(s_prof[g])
        rows = rnode[g * RG:(g + 1) * RG]
        valid_r = rows >= 0
        rr = np.where(valid_r, rows, 0)
        deg = np.where(valid_r, counts[rr], 0)
        start = cum[rr]
        sl = np.arange(S)
        eix = start[:, None] + sl[None, :]
        vmask = sl[None, :] < deg[:, None]
        eid = order_e[np.clip(eix, 0, n_edges - 1)]

        kg = key_f[eid]                        # [RG, S, NF]
        kp = kpad[rr] * valid_r[:, None]
        kg = np.where(vmask[..., None], kg, kp[:, None, :])
        vg = val_f[eid]
        vg[~vmask] = 0.0
        qg = q_cat[rr] * valid_r[:, None]

        kg = kg.reshape(N_CORES, P, S, NF)
        vg = vg.reshape(N_CORES, P, S, NF)
        qg = qg.reshape(N_CORES, P, NF)
        kT_all[:, :, off:off + S * P] = kg.transpose(0, 3, 2, 1).reshape(
            N_CORES, NF, S * P).astype(bf16)
        v_all[:, :, off:off + S * P] = vg.reshape(
            N_CORES, P, S * NF).astype(bf16)
        qT_all[:, g] = qg.transpose(0, 2, 1).astype(bf16)
        off += S * P

    hm = np.zeros((NF, H), np.float32)
    for h in range(H):
        hm[h * HS:(h + 1) * HS, h] = 1.0
    hm = hm.astype(bf16)

    in_maps = []
    for c in range(N_CORES):
        in_maps.append({
            "kT": kT_all[c], "v": v_all[c], "qT": qT_all[c], "hm": hm,
        })
    return in_maps


def _assemble(results, plan):
    G = plan["G"]
    n_nodes = plan["n_nodes"]
    rnode = plan["rnode"].reshape(G, N_CORES, P)
    out = np.zeros((n_nodes, NF), np.float32)
    for c in range(N_CORES):
        arr = np.asarray(results[c]["out"], dtype=np.float32).reshape(
            G * P, NF)
        idx = rnode[:, c, :].ravel()
        m = idx >= 0
        out[idx[m]] = arr[m]
    return out.reshape(n_nodes, NF // 4, 4)


def _get_nc(s_prof):
    key = tuple(int(s) for s in s_prof)
    if key not in _CACHE:
        _CACHE[key] = build_nc(s_prof)
    return _CACHE[key]


def _run(inputs, trace=False, **spmd_kwargs):
    global LAST_RESULTS
    from concourse.bass_utils import run_bass_kernel_spmd

    n_nodes = np.asarray(inputs["query_0"]).shape[0]
    plan = _plan(inputs["edge_index"], n_nodes)
    nc = _get_nc(plan["s_prof"])
    in_maps = _prep_inputs(
        inputs["value"], inputs["key"], inputs["query_0"], inputs["query_1"],
        plan)
    res = run_bass_kernel_spmd(
        nc, in_maps, list(range(N_CORES)), trace=trace, **spmd_kwargs)
    LAST_RESULTS = res
    return _assemble(res.results, plan)


def kernel(value, key, query_0, query_1, edge_index):
    return _run({
        "value": value, "key": key, "query_0": query_0,
        "query_1": query_1, "edge_index": edge_index,
    })


# revision 4
# speedup vs baseline: 1.0965x; 1.0368x over previous
"""SE(3) attention block (GNN message passing) on 8 Trainium2 NeuronCores.

Strategy (slot format, v3.1)
----------------------------
Nodes are sorted by in-degree (host) and cut into tiles of 128 nodes.
Tiles are grouped 8 at a time (one tile per core, SPMD) and every tile in
a group is padded to the group's max degree S, so all 8 cores execute an
identical per-tile shape profile.  Because nodes in a tile have nearly
equal degree (sorted), slot padding is only ~2-3% of E.

Each node-row owns its incoming edges as "slots" 0..S-1, so the segment
softmax and the weighted aggregation become *free-axis* operations on the
node-partitioned tile -- no one-hot matrices, no per-edge gathered query,
no cross-device collectives:

  1. prodT[f, (s, n)] = kT * qT (DVE, bf16 2x; q broadcast over slots via
     a 0-stride middle AP dim -- queries ship once per node, not per edge)
  2. scores[n, (s, h)]: S head-mask matmuls (PE, otherwise idle)
  3. exw[n, (s, f)] = exp(scores / sqrt(NF)) widened to 16 feats/head in
     one ACT op (replicated 0-stride read straight from PSUM); a second
     tiny ACT exp writes the narrow (h, s)-major copy for ssum
  4. evex[n, (s, f)] = v * exw (DVE, bf16 2x)
  5. agg[n, f]: dense pairwise slot-fold tree (bf16 2x adds; first levels
     on GPSIMD which is otherwise idle; final add in f32)
  6. ssum[n, h] = contiguous reduce of the (h, s)-major ex (f32),
     inv = 1/ssum (no eps: pad-slot design keeps ssum > 0)
  7. out = agg * inv (bf16 out)

Padding slots carry k_pad = -C * q_h / |q_h|^2 per head-block so every
head scores -C (ex ~ 1e-13) and v_pad = 0 -- no masks needed.
"""

import math
import numpy as np

# ---------------------------------------------------------------- constants
N_CORES = 8
P = 128                 # partitions / nodes per tile
H = 8                   # heads
NF = 128                # features per edge (32*4)
HS = NF // H            # head size (16)
INV_SQRT_NF = 1.0 / math.sqrt(NF)
C_PAD = 345.0           # pad-slot per-head score magnitude (scaled: ~-30.5)

GPS_LEVELS = 1          # how many leading fold levels run on GPSIMD

_CACHE = {}
LAST_RESULTS = None     # BassKernelResults of the most recent run (for test.py)


# ---------------------------------------------------------------- device IR
def build_nc(s_prof, gps_levels=GPS_LEVELS):
    """Per-core Bass/Tile program; identical on all 8 cores (SPMD)."""
    from contextlib import ExitStack

    import concourse.bacc as bacc
    import concourse.mybir as mybir
    from concourse.tile import TileContext

    f32 = mybir.dt.float32
    bf16 = mybir.dt.bfloat16
    G = len(s_prof)
    W = int(sum(s_prof)) * P

    nc = bacc.Bacc("TRN2", target_bir_lowering=False, debug=False)
    kT_d = nc.dram_tensor("kT", [P, W], bf16, kind="ExternalInput")
    v_d = nc.dram_tensor("v", [P, W], bf16, kind="ExternalInput")
    qT_d = nc.dram_tensor("qT", [G, P, P], bf16, kind="ExternalInput")
    hm_d = nc.dram_tensor("hm", [P, H], bf16, kind="ExternalInput")
    out_d = nc.dram_tensor("out", [G, P, P], bf16, kind="ExternalOutput")

    with TileContext(nc) as tc, ExitStack() as ctx:
        singles = ctx.enter_context(tc.tile_pool(name="singles", bufs=1))
        inp = ctx.enter_context(tc.tile_pool(name="inp", bufs=4))
        mid = ctx.enter_context(tc.tile_pool(name="mid", bufs=3))
        sml = ctx.enter_context(tc.tile_pool(name="sml", bufs=4))
        ps = ctx.enter_context(tc.tile_pool(name="ps", bufs=6, space="PSUM"))

        hm = singles.tile([P, H], bf16)
        nc.sync.dma_start(out=hm[:], in_=hm_d[:, :])

        off = 0
        for g in range(G):
            S = int(s_prof[g])
            Wt = S * P
            kT = inp.tile([P, Wt], bf16, tag="kT")
            nc.sync.dma_start(out=kT[:], in_=kT_d[:, off:off + Wt])
            v = inp.tile([P, Wt], bf16, tag="v")
            nc.sync.dma_start(out=v[:], in_=v_d[:, off:off + Wt])
            qT = sml.tile([P, P], bf16, tag="qT")
            nc.sync.dma_start(out=qT[:], in_=qT_d[g])
            off += Wt

            # 1. prodT[f, (s, n)] = kT * qT (q broadcast over slots)
            prodT = mid.tile([P, Wt], bf16, tag="prodT")
            nc.vector.tensor_tensor(
                out=prodT[:].rearrange("p (s n) -> p s n", s=S),
                in0=kT[:].rearrange("p (s n) -> p s n", s=S),
                in1=qT[:, :].unsqueeze(1).broadcast_to([P, S, P]),
                op=mybir.AluOpType.mult)

            # 2. per-slot head-mask matmuls -> scores [n, (s, h)] in PSUM
            sc = ps.tile([P, S * H], f32, tag="sc")
            for s in range(S):
                nc.tensor.matmul(
                    out=sc[:, s * H:(s + 1) * H],
                    lhsT=prodT[:, s * P:(s + 1) * P], rhs=hm[:],
                    start=True, stop=True)

            # 3a. widened exp straight from PSUM (one ACT op)
            exw = mid.tile([P, Wt], bf16, tag="exw")
            nc.scalar.activation(
                out=exw[:].rearrange("p (s h j) -> p s h j", s=S, h=H),
                in_=sc[:].rearrange("p (s h) -> p s h", s=S)
                    .to_broadcast([P, S, H, HS]),
                func=mybir.ActivationFunctionType.Exp, scale=INV_SQRT_NF)
            # 3b. narrow (h, s)-major exp for the softmax denominator
            ex = sml.tile([P, S * H], bf16, tag="ex")
            nc.scalar.activation(
                out=ex[:].rearrange("p (h s) -> p h s", h=H),
                in_=sc[:].rearrange("p (s h) -> p h s", s=S),
                func=mybir.ActivationFunctionType.Exp, scale=INV_SQRT_NF)

            # 4. evex = v * exw
            evex = mid.tile([P, Wt], bf16, tag="evex")
            nc.vector.tensor_tensor(
                out=evex[:], in0=v[:], in1=exw[:], op=mybir.AluOpType.mult)

            # 5. agg: dense pairwise slot-fold tree; first levels on GPSIMD
            cur, src, lvl = S, evex, 0
            while cur > 2:
                nxt = cur // 2
                eng = nc.gpsimd if lvl < gps_levels else nc.vector
                dst = mid.tile([P, nxt * P], bf16, tag=f"fold{lvl}")
                eng.tensor_tensor(
                    out=dst[:], in0=src[:, 0:nxt * P],
                    in1=src[:, nxt * P:2 * nxt * P], op=mybir.AluOpType.add)
                if cur - 2 * nxt:   # odd: carry last slot into block 0
                    nc.vector.tensor_tensor(
                        out=dst[:, 0:P], in0=dst[:, 0:P],
                        in1=src[:, 2 * nxt * P:(2 * nxt + 1) * P],
                        op=mybir.AluOpType.add)
                cur, src, lvl = nxt, dst, lvl + 1
            agg = sml.tile([P, P], f32, tag="agg")
            if cur == 2:
                nc.vector.tensor_tensor(
                    out=agg[:], in0=src[:, 0:P], in1=src[:, P:2 * P],
                    op=mybir.AluOpType.add)
            else:
                nc.vector.tensor_scalar(
                    out=agg[:], in0=src[:, 0:P], scalar1=0.0, scalar2=None,
                    op0=mybir.AluOpType.add)

            # 6. ssum (contiguous reduce over s), inv = 1/ssum
            ssum = sml.tile([P, H], f32, tag="ssum")
            nc.vector.tensor_reduce(
                out=ssum[:], in_=ex[:].rearrange("p (h s) -> p h s", h=H),
                axis=mybir.AxisListType.X, op=mybir.AluOpType.add)
            inv = sml.tile([P, H], f32, tag="inv")
            nc.vector.reciprocal(out=inv[:], in_=ssum[:])

            # 7. normalize -> bf16 out
            outb = sml.tile([P, P], bf16, tag="outb")
            nc.vector.tensor_tensor(
                out=outb[:].rearrange("p (h j) -> p h j", h=H),
                in0=agg[:].rearrange("p (h j) -> p h j", h=H),
                in1=inv[:].to_broadcast([P, H, HS]),
                op=mybir.AluOpType.mult)
            nc.sync.dma_start(out=out_d[g], in_=outb[:])
    nc.compile()
    return nc


# ------------------------------------------------------------ host plumbing
def _plan(edge_index, n_nodes):
    """Degree-sorted tile plan shared by all cores."""
    dst = np.asarray(edge_index)[1].astype(np.int64).ravel()
    n_edges = dst.shape[0]
    counts = np.bincount(dst, minlength=n_nodes)
    order_e = np.argsort(dst, kind="stable")
    cum = np.zeros(n_nodes + 1, np.int64)
    cum[1:] = np.cumsum(counts)
    nperm = np.argsort(-counts, kind="stable")

    n_tiles = -(-n_nodes // P)
    G = -(-n_tiles // N_CORES)
    rows_total = G * N_CORES * P
    rnode = np.full(rows_total, -1, np.int64)
    rnode[:n_nodes] = nperm

    deg_pad = np.zeros(rows_total, np.int64)
    deg_pad[:n_nodes] = counts[nperm]
    s_prof = deg_pad.reshape(G, N_CORES * P).max(axis=1)
    s_prof = np.maximum(s_prof, 4).astype(np.int64)
    return dict(counts=counts, order_e=order_e, cum=cum, rnode=rnode,
                s_prof=s_prof, G=G, n_edges=n_edges, n_nodes=n_nodes)


def _prep_inputs(value, key, query_0, query_1, plan):
    import ml_dtypes
    bf16 = ml_dtypes.bfloat16

    G = plan["G"]
    s_prof = plan["s_prof"]
    rnode = plan["rnode"]
    counts, order_e, cum = plan["counts"], plan["order_e"], plan["cum"]
    n_edges = plan["n_edges"]
    n_nodes = plan["n_nodes"]

    key_f = np.asarray(key, dtype=np.float32).reshape(n_edges, NF)
    val_f = np.asarray(value, dtype=np.float32).reshape(n_edges, NF)
    q_cat = np.concatenate(
        [np.asarray(query_0, np.float32), np.asarray(query_1, np.float32)],
        axis=-1).reshape(n_nodes, NF)
    # pad slots must score ~-C in EVERY head (scores are per-head dots over
    # 16 features), so normalize q per head-block
    qh = q_cat.reshape(n_nodes, H, HS)
    qh2 = np.einsum("nhj,nhj->nh", qh, qh)
    kpad = (-C_PAD * qh / np.maximum(qh2, 0.1)[:, :, None]).reshape(
        n_nodes, NF)

    W = int(s_prof.sum()) * P
    kT_all = np.empty((N_CORES, P, W), bf16)
    v_all = np.empty((N_CORES, P, W), bf16)
    qT_all = np.empty((N_CORES, G, P, P), bf16)

    off = 0
    RG = N_CORES * P       # rows per group
    for g in range(G):
        S = int(s_prof[g])
        rows = rnode[g * RG:(g + 1) * RG]
        valid_r = rows >= 0
        rr = np.where(valid_r, rows, 0)
        deg = np.where(valid_r, counts[rr], 0)
        start = cum[rr]
        sl = np.arange(S)
        eix = start[:, None] + sl[None, :]
        vmask = sl[None, :] < deg[:, None]
        eid = order_e[np.clip(eix, 0, n_edges - 1)]

        kg = key_f[eid]                        # [RG, S, NF]
        kp = kpad[rr] * valid_r[:, None]
        kg = np.where(vmask[..., None], kg, kp[:, None, :])
        vg = val_f[eid]
        vg[~vmask] = 0.0
        qg = q_cat[rr] * valid_r[:, None]

        kg = kg.reshape(N_CORES, P, S, NF)
        vg = vg.reshape(N_CORES, P, S, NF)
        qg = qg.reshape(N_CORES, P, NF)
        kT_all[:, :, off:off + S * P] = kg.transpose(0, 3, 2, 1).reshape(
            N_CORES, NF, S * P).astype(bf16)
        v_all[:, :, off:off + S * P] = vg.reshape(
            N_CORES, P, S * NF).astype(bf16)
        qT_all[:, g] = qg.transpose(0, 2, 1).astype(bf16)
        off += S * P

    hm = np.zeros((NF, H), np.float32)
    for h in range(H):
        hm[h * HS:(h + 1) * HS, h] = 1.0
    hm = hm.astype(bf16)

    in_maps = []
    for c in range(N_CORES):
        in_maps.append({
            "kT": kT_all[c], "v": v_all[c], "qT": qT_all[c], "hm": hm,
        })
    return in_maps


def _assemble(results, plan):
    G = plan["G"]
    n_nodes = plan["n_nodes"]
    rnode = plan["rnode"].reshape(G, N_CORES, P)
    out = np.zeros((n_nodes, NF), np.float32)
    for c in range(N_CORES):
        arr = np.asarray(results[c]["out"], dtype=np.float32).reshape(
            G * P, NF)
        idx = rnode[:, c, :].ravel()
        m = idx >= 0
        out[idx[m]] = arr[m]
    return out.reshape(n_nodes, NF // 4, 4)


def _get_nc(s_prof, gps_levels=GPS_LEVELS):
    key = (tuple(int(s) for s in s_prof), gps_levels)
    if key not in _CACHE:
        _CACHE[key] = build_nc(s_prof, gps_levels)
    return _CACHE[key]


def _run(inputs, trace=False, gps_levels=GPS_LEVELS, **spmd_kwargs):
    global LAST_RESULTS
    from concourse.bass_utils import run_bass_kernel_spmd

    n_nodes = np.asarray(inputs["query_0"]).shape[0]
    plan = _plan(inputs["edge_index"], n_nodes)
    nc = _get_nc(plan["s_prof"], gps_levels)
    in_maps = _prep_inputs(
        inputs["value"], inputs["key"], inputs["query_0"], inputs["query_1"],
        plan)
    res = run_bass_kernel_spmd(
        nc, in_maps, list(range(N_CORES)), trace=trace, **spmd_kwargs)
    LAST_RESULTS = res
    return _assemble(res.results, plan)


def kernel(value, key, query_0, query_1, edge_index):
    return _run({
        "value": value, "key": key, "query_0": query_0,
        "query_1": query_1, "edge_index": edge_index,
    })


# revision 5
# speedup vs baseline: 1.4236x; 1.2983x over previous
"""SE(3) attention block (GNN message passing) on 8 Trainium2 NeuronCores.

Strategy (slot format, v3.2)
----------------------------
Nodes are sorted by in-degree (host) and cut into tiles of 128 nodes.
Tiles are grouped into batches of 8*b tiles (b per core, SPMD-identical
shapes) padded to the batch max degree S; degree sorting keeps slot
padding at ~2-4% of E.

Each node-row owns its incoming edges as "slots" 0..S-1, so the segment
softmax and the weighted aggregation become *free-axis* operations on the
node-partitioned tile -- no one-hot matrices, no per-edge gathered query,
no cross-device collectives.  Per batch (b tiles, S slots):

  1. prodT[f, (t, s, n)] = kT * qT (DVE, bf16 2x; q broadcast over slots
     via a 0-stride AP dim -- queries ship once per node, not per edge)
  2. scores[n, (t, s, h)]: b*S head-mask matmuls (PE, otherwise idle)
  3. exw[n, (t, s, f)] = exp(scores / sqrt(NF)) widened 16x in one ACT op
     per tile (replicated 0-stride read straight from PSUM); a second
     tiny ACT exp writes the narrow (h, s)-major copy for ssum
  4. evex = v * exw (DVE, bf16 2x, whole batch)
  5. agg[n, (t, f)]: dense pairwise slot-fold tree (bf16 2x adds, final
     add in f32), one instruction per level per batch
  6. ssum[n, (t, h)] = contiguous reduce of the narrow ex (f32),
     inv = 1/ssum (no eps: pad-slot design keeps ssum > 0)
  7. out = agg * inv (bf16 out)

Batching equal-S tiles keeps instruction counts (and sequencer/semaphore
overhead) low.  Input DMAs issue from the GPSIMD queue (25ns/issue vs
565ns on sync).  GPSIMD compute is intentionally unused: concurrent
GPSIMD tensor ops slow DVE ops by 2.5-3x (measured SBUF contention).

Padding slots carry k_pad = -C * q_h / |q_h|^2 per head-block so every
head scores -C (ex ~ 1e-13) and v_pad = 0 -- no masks needed.
"""

import math
import numpy as np

# ---------------------------------------------------------------- constants
N_CORES = 8
P = 128                 # partitions / nodes per tile
H = 8                   # heads
NF = 128                # features per edge (32*4)
HS = NF // H            # head size (16)
INV_SQRT_NF = 1.0 / math.sqrt(NF)
C_PAD = 345.0           # pad-slot per-head score magnitude (scaled: ~-30.5)
MAX_BATCH_SLOTS = 44    # b*S cap (PSUM bank + SBUF budget)
MAX_B = 6               # tiles per core per batch cap

_CACHE = {}
LAST_RESULTS = None     # BassKernelResults of the most recent run (for test.py)


# ---------------------------------------------------------------- device IR
def build_nc(batch_prof):
    """Per-core Bass/Tile program; identical on all 8 cores (SPMD).

    batch_prof: tuple of (S, b) per batch.
    """
    from contextlib import ExitStack

    import concourse.bacc as bacc
    import concourse.mybir as mybir
    from concourse.tile import TileContext

    f32 = mybir.dt.float32
    bf16 = mybir.dt.bfloat16
    W = int(sum(S * b for S, b in batch_prof)) * P
    Wq = int(sum(b for S, b in batch_prof)) * P

    nc = bacc.Bacc("TRN2", target_bir_lowering=False, debug=False)
    kT_d = nc.dram_tensor("kT", [P, W], bf16, kind="ExternalInput")
    v_d = nc.dram_tensor("v", [P, W], bf16, kind="ExternalInput")
    qT_d = nc.dram_tensor("qT", [P, Wq], bf16, kind="ExternalInput")
    hm_d = nc.dram_tensor("hm", [P, H], bf16, kind="ExternalInput")
    out_d = nc.dram_tensor("out", [P, Wq], bf16, kind="ExternalOutput")

    with TileContext(nc) as tc, ExitStack() as ctx:
        singles = ctx.enter_context(tc.tile_pool(name="singles", bufs=1))
        inp = ctx.enter_context(tc.tile_pool(name="inp", bufs=3))
        mid = ctx.enter_context(tc.tile_pool(name="mid", bufs=2))
        sml = ctx.enter_context(tc.tile_pool(name="sml", bufs=3))
        ps = ctx.enter_context(tc.tile_pool(name="ps", bufs=4, space="PSUM"))

        hm = singles.tile([P, H], bf16)
        nc.sync.dma_start(out=hm[:], in_=hm_d[:, :])

        off = 0
        offq = 0
        for S, b in batch_prof:
            S, b = int(S), int(b)
            Wt = b * S * P
            Wqt = b * P
            kT = inp.tile([P, Wt], bf16, tag="kT")
            nc.gpsimd.dma_start(out=kT[:], in_=kT_d[:, off:off + Wt])
            v = inp.tile([P, Wt], bf16, tag="v")
            nc.gpsimd.dma_start(out=v[:], in_=v_d[:, off:off + Wt])
            qT = sml.tile([P, Wqt], bf16, tag="qT")
            nc.gpsimd.dma_start(out=qT[:], in_=qT_d[:, offq:offq + Wqt])

            # 1. prodT[f, (t, s, n)] = kT * qT (q broadcast over slots)
            prodT = mid.tile([P, Wt], bf16, tag="prodT")
            nc.vector.tensor_tensor(
                out=prodT[:].rearrange("p (t s n) -> p t s n", t=b, s=S),
                in0=kT[:].rearrange("p (t s n) -> p t s n", t=b, s=S),
                in1=qT[:].rearrange("p (t n) -> p t n", t=b)
                    .unsqueeze(2).broadcast_to([P, b, S, P]),
                op=mybir.AluOpType.mult)

            # 2. per-(tile,slot) head-mask matmuls -> scores in PSUM
            sc = ps.tile([P, b * S * H], f32, tag="sc")
            for ts in range(b * S):
                nc.tensor.matmul(
                    out=sc[:, ts * H:(ts + 1) * H],
                    lhsT=prodT[:, ts * P:(ts + 1) * P], rhs=hm[:],
                    start=True, stop=True)

            # 3. exps from PSUM, per tile: widened + narrow (h, s)-major
            exw = mid.tile([P, Wt], bf16, tag="exw")
            ex = sml.tile([P, b * S * H], bf16, tag="ex")
            for t in range(b):
                nc.scalar.activation(
                    out=exw[:, t * S * P:(t + 1) * S * P]
                        .rearrange("p (s h j) -> p s h j", s=S, h=H),
                    in_=sc[:, t * S * H:(t + 1) * S * H]
                        .rearrange("p (s h) -> p s h", s=S)
                        .to_broadcast([P, S, H, HS]),
                    func=mybir.ActivationFunctionType.Exp, scale=INV_SQRT_NF)
                nc.scalar.activation(
                    out=ex[:, t * S * H:(t + 1) * S * H]
                        .rearrange("p (h s) -> p h s", h=H),
                    in_=sc[:, t * S * H:(t + 1) * S * H]
                        .rearrange("p (s h) -> p h s", s=S),
                    func=mybir.ActivationFunctionType.Exp, scale=INV_SQRT_NF)

            # 4. evex = v * exw (whole batch)
            evex = mid.tile([P, Wt], bf16, tag="evex")
            nc.vector.tensor_tensor(
                out=evex[:], in0=v[:], in1=exw[:], op=mybir.AluOpType.mult)

            # 5. agg: dense pairwise slot-fold tree (per level per batch)
            cur, src, lvl = S, evex, 0
            while cur > 2:
                nxt = cur // 2
                dst = mid.tile([P, b * nxt * P], bf16, tag=f"fold{lvl}")
                sv = src[:].rearrange("p (t s n) -> p t s n", t=b, s=cur)
                dv = dst[:].rearrange("p (t s n) -> p t s n", t=b, s=nxt)
                nc.vector.tensor_tensor(
                    out=dv, in0=sv[:, :, 0:nxt, :], in1=sv[:, :, nxt:2 * nxt, :],
                    op=mybir.AluOpType.add)
                if cur - 2 * nxt:   # odd: carry last slot into slot 0
                    nc.vector.tensor_tensor(
                        out=dv[:, :, 0:1, :], in0=dv[:, :, 0:1, :],
                        in1=sv[:, :, 2 * nxt:2 * nxt + 1, :],
                        op=mybir.AluOpType.add)
                cur, src, lvl = nxt, dst, lvl + 1
            agg = sml.tile([P, b * P], f32, tag="agg")
            sv = src[:].rearrange("p (t s n) -> p t s n", t=b, s=cur)
            av = agg[:].rearrange("p (t n) -> p t n", t=b).unsqueeze(2)
            if cur == 2:
                nc.vector.tensor_tensor(
                    out=av, in0=sv[:, :, 0:1, :], in1=sv[:, :, 1:2, :],
                    op=mybir.AluOpType.add)
            else:
                nc.vector.tensor_scalar(
                    out=av, in0=sv[:, :, 0:1, :], scalar1=0.0, scalar2=None,
                    op0=mybir.AluOpType.add)

            # 6. ssum (contiguous reduce over s), inv = 1/ssum
            ssum = sml.tile([P, b * H], f32, tag="ssum")
            nc.vector.tensor_reduce(
                out=ssum[:],
                in_=ex[:].rearrange("p (t h s) -> p t h s", t=b, h=H),
                axis=mybir.AxisListType.X, op=mybir.AluOpType.add)
            inv = sml.tile([P, b * H], f32, tag="inv")
            nc.vector.reciprocal(out=inv[:], in_=ssum[:])

            # 7. normalize -> bf16 out
            outb = sml.tile([P, b * P], bf16, tag="outb")
            nc.vector.tensor_tensor(
                out=outb[:].rearrange("p (t h j) -> p t h j", t=b, h=H),
                in0=agg[:].rearrange("p (t h j) -> p t h j", t=b, h=H),
                in1=inv[:].rearrange("p (t h) -> p t h", t=b)
                    .to_broadcast([P, b, H, HS]),
                op=mybir.AluOpType.mult)
            nc.sync.dma_start(out=out_d[:, offq:offq + Wqt], in_=outb[:])
            off += Wt
            offq += Wqt
    nc.compile()
    return nc


# ------------------------------------------------------------ host plumbing
def _plan(edge_index, n_nodes):
    """Degree-sorted batched tile plan shared by all cores."""
    dst = np.asarray(edge_index)[1].astype(np.int64).ravel()
    n_edges = dst.shape[0]
    counts = np.bincount(dst, minlength=n_nodes)
    order_e = np.argsort(dst, kind="stable")
    cum = np.zeros(n_nodes + 1, np.int64)
    cum[1:] = np.cumsum(counts)
    nperm = np.argsort(-counts, kind="stable")

    n_tiles = -(-n_nodes // P)
    deg_desc = np.zeros(n_tiles * P, np.int64)
    deg_desc[:n_nodes] = counts[nperm]

    batches = []            # (S, b, tile_start)
    t = 0
    while t < n_tiles:
        S = max(int(deg_desc[t * P]), 4)
        rem_groups = -(-(n_tiles - t) // N_CORES)
        b = max(1, min(MAX_B, MAX_BATCH_SLOTS // S, rem_groups))
        batches.append((S, b, t))
        t += N_CORES * b

    total_tiles = sum(N_CORES * b for S, b, _ in batches)
    rnode = np.full(total_tiles * P, -1, np.int64)
    rnode[:n_nodes] = nperm
    return dict(counts=counts, order_e=order_e, cum=cum, rnode=rnode,
                batches=batches, n_edges=n_edges, n_nodes=n_nodes)


def _prep_inputs(value, key, query_0, query_1, plan):
    import ml_dtypes
    bf16 = ml_dtypes.bfloat16

    batches = plan["batches"]
    rnode = plan["rnode"]
    counts, order_e, cum = plan["counts"], plan["order_e"], plan["cum"]
    n_edges = plan["n_edges"]
    n_nodes = plan["n_nodes"]

    key_f = np.asarray(key, dtype=np.float32).reshape(n_edges, NF)
    val_f = np.asarray(value, dtype=np.float32).reshape(n_edges, NF)
    q_cat = np.concatenate(
        [np.asarray(query_0, np.float32), np.asarray(query_1, np.float32)],
        axis=-1).reshape(n_nodes, NF)
    # pad slots must score ~-C in EVERY head (scores are per-head dots over
    # 16 features), so normalize q per head-block
    qh = q_cat.reshape(n_nodes, H, HS)
    qh2 = np.einsum("nhj,nhj->nh", qh, qh)
    kpad = (-C_PAD * qh / np.maximum(qh2, 0.1)[:, :, None]).reshape(
        n_nodes, NF)

    W = sum(S * b for S, b, _ in batches) * P
    Wq = sum(b for S, b, _ in batches) * P
    kT_all = np.empty((N_CORES, P, W), bf16)
    v_all = np.empty((N_CORES, P, W), bf16)
    qT_all = np.empty((N_CORES, P, Wq), bf16)

    off = 0
    offq = 0
    for S, b, t0 in batches:
        nb = N_CORES * b * P
        rows = rnode[t0 * P:t0 * P + nb]
        valid_r = rows >= 0
        rr = np.where(valid_r, rows, 0)
        deg = np.where(valid_r, counts[rr], 0)
        start = cum[rr]
        sl = np.arange(S)
        eix = start[:, None] + sl[None, :]
        vmask = sl[None, :] < deg[:, None]
        eid = order_e[np.clip(eix, 0, n_edges - 1)]

        kg = key_f[eid]                        # [8*b*128, S, NF]
        kp = kpad[rr] * valid_r[:, None]
        kg = np.where(vmask[..., None], kg, kp[:, None, :])
        vg = val_f[eid]
        vg[~vmask] = 0.0
        qg = q_cat[rr] * valid_r[:, None]

        # [core, t, n, S, NF]
        kg = kg.reshape(N_CORES, b, P, S, NF)
        vg = vg.reshape(N_CORES, b, P, S, NF)
        qg = qg.reshape(N_CORES, b, P, NF)
        # kT: [f, (t, s, n)]
        kT_all[:, :, off:off + b * S * P] = kg.transpose(0, 4, 1, 3, 2).reshape(
            N_CORES, NF, b * S * P).astype(bf16)
        # v: [n, (t, s, f)]
        v_all[:, :, off:off + b * S * P] = vg.transpose(0, 2, 1, 3, 4).reshape(
            N_CORES, P, b * S * NF).astype(bf16)
        # qT: [f, (t, n)]
        qT_all[:, :, offq:offq + b * P] = qg.transpose(0, 3, 1, 2).reshape(
            N_CORES, NF, b * P).astype(bf16)
        off += b * S * P
        offq += b * P

    hm = np.zeros((NF, H), np.float32)
    for h in range(H):
        hm[h * HS:(h + 1) * HS, h] = 1.0
    hm = hm.astype(bf16)

    in_maps = []
    for c in range(N_CORES):
        in_maps.append({
            "kT": kT_all[c], "v": v_all[c], "qT": qT_all[c], "hm": hm,
        })
    return in_maps


def _assemble(results, plan):
    batches = plan["batches"]
    n_nodes = plan["n_nodes"]
    rnode = plan["rnode"]
    out = np.zeros((n_nodes, NF), np.float32)
    for c in range(N_CORES):
        arr = np.asarray(results[c]["out"], dtype=np.float32)  # [P, Wq]
        offq = 0
        for S, b, t0 in batches:
            blk = arr[:, offq:offq + b * P].reshape(P, b, NF)
            rows = rnode[(t0 + c * b) * P:(t0 + (c + 1) * b) * P].reshape(
                b, P)
            for t in range(b):
                idx = rows[t]
                m = idx >= 0
                out[idx[m]] = blk[m, t]
            offq += b * P
    return out.reshape(n_nodes, NF // 4, 4)


def _get_nc(batch_prof):
    key = tuple(batch_prof)
    if key not in _CACHE:
        _CACHE[key] = build_nc(batch_prof)
    return _CACHE[key]


def _run(inputs, trace=False, **spmd_kwargs):
    global LAST_RESULTS
    from concourse.bass_utils import run_bass_kernel_spmd

    n_nodes = np.asarray(inputs["query_0"]).shape[0]
    plan = _plan(inputs["edge_index"], n_nodes)
    batch_prof = tuple((int(S), int(b)) for S, b, _ in plan["batches"])
    nc = _get_nc(batch_prof)
    in_maps = _prep_inputs(
        inputs["value"], inputs["key"], inputs["query_0"], inputs["query_1"],
        plan)
    res = run_bass_kernel_spmd(
        nc, in_maps, list(range(N_CORES)), trace=trace, **spmd_kwargs)
    LAST_RESULTS = res
    return _assemble(res.results, plan)


def kernel(value, key, query_0, query_1, edge_index):
    return _run({
        "value": value, "key": key, "query_0": query_0,
        "query_1": query_1, "edge_index": edge_index,
    })
